# revision 40
# baseline (speedup 1.0000x reference)
"""DSH loss kernel for Trainium2 (8 NeuronCores, Bass/Tile).

Math (reference):
    U[ind] = u; Y[ind] = y
    raw[b,n]  = ||u_b||^2 - 2 u_b.U_n + ||U_n||^2          (>= 0 mathematically)
    dist      = max(raw, 0)
    match[b,n]= y_b . Y_n          (integer >= 0)
    m         = (match == 0)       ("mismatch" mask, statistically ~never 1)
    loss1 = mean( (1-m)*0.5*dist + m*0.5*relu(M - dist) )
    loss2 = ALPHA * mean(|1 - sign(u)|)

Decomposition (v6):
    2*B*N*loss1 = S_raw + sum_{m=1} [ relu(M - raw) - raw ]
      S_raw factorizes (N*sum(u_sq) + B*sum(U_sq) - 2*colsum(u).colsum(U))
      -> exact fp64 on host. Distances never touch the device: the
      correction only needs the LOCATIONS of match==0 pairs, found from
      the labels alone (half the matmul work of computing distances).

Device work per core (shard = 12500 gallery rows, padded to 12544):
    match[b,n] = y_b . Y_n, one bf16 K=128 matmul per 128-row gallery
    tile (binary labels are bf16-exact; fp32 PSUM accumulation gives
    exact integer counts). Measured per-MM cost on this part is ~320ns
    (K=128, free=512) regardless of dtype; fp8 / DoubleRow / smaller K
    are all equal or slower, out free > 512 is rejected by ISA checks,
    and LDWEIGHTS is re-emitted per matmul (ldw-opt off), so 98
    matmuls x ~320ns is the PE floor. A short warm-up burst of y x y
    dummy matmuls absorbs the PE p-state ramp inside the gallery-DMA
    window, which keeps the real stream gap-free.

    Zero-match detector, one probe per pair over 49 super-tiles
    [128, 1024] (2 PSUM banks = 2 gallery tiles x 512 batch):
      - ScalarE: activation Relu(0.5 - s) + accum_out column
        (> 0.25 flags a zero-match); ~1396ns/super incl accumulator
        read.
      - VectorE: tensor_reduce min column (< 0.5 flags); ~1218ns/super.
    Supers are assigned greedily by those measured costs so both
    engines finish together (~32us); ScalarE writes its relu output
    in-place over the PSUM input. Gallery DMA rides the sync ring
    alone (ample bandwidth; keeping the gpsimd ring's software-DGE
    generation and drains off the critical path measured ~2us faster
    than a two-ring split), small chunks first so tile 0 lands ASAP.
    Flagged (gallery row, batch-all) candidates are re-checked exactly
    on host in fp64 (normally there are none).
"""

import numpy as np
import ml_dtypes

import concourse.bass as bass
import concourse.mybir as mybir
import concourse.tile as tile
from concourse import bacc
from concourse.bass_utils import run_bass_kernel_spmd

# Problem constants (hardcoded per harness contract)
B = 512
BIT = 64
C = 100
N = 100000
N_CORES = 8
N_SH = N // N_CORES          # 12500
M_MARGIN = 2.0 * BIT         # 128.0
ALPHA = 0.1
P_TILE = 128                 # gallery rows per tile

BF16 = ml_dtypes.bfloat16

# measured EW instruction cost model (ns): fixed + var * free_elems
_ACT_FIX, _ACT_VAR = 536.0, 0.841     # incl accumulator read
_DVE_FIX, _DVE_VAR = 150.0, 1.039


def _layout(n_sh: int):
    """Units of 2 gallery tiles ([128,1024] super-tiles, 2 PSUM banks,
    4-deep ring). Returns (n_pad, units): units = [(t0, ntiles), ...]."""
    n_tiles = -(-n_sh // P_TILE)
    n_pad = n_tiles * P_TILE
    units = []
    t = 0
    while t < n_tiles:
        w = min(2, n_tiles - t)
        if w == 2 and t + 2 == n_tiles:
            # final pair as two singles: the post-stream EW tail is one
            # 512-wide probe per engine instead of a full super
            units.append((t, 1))
            units.append((t + 1, 1))
            t += 2
        else:
            units.append((t, w))
            t += w
    return n_pad, units


def _schedule(n_sh: int):
    """Greedy engine assignment per unit, balancing the measured cost
    model. Returns (kinds, units): accD col == unit index."""
    n_pad, units = _layout(n_sh)
    kinds = []
    t_act = t_dve = 0.0
    for t0, w in units:
        e = 512.0 * w
        ca = _ACT_FIX + _ACT_VAR * e
        cd = _DVE_FIX + _DVE_VAR * e
        if t_act + ca <= t_dve + cd:
            kinds.append("ACT")
            t_act += ca
        else:
            kinds.append("DVE")
            t_dve += cd
    return kinds, units


def _build_program(n_sh: int):
    fp32 = mybir.dt.float32
    bf16 = mybir.dt.bfloat16
    nc = bacc.Bacc("TRN2", target_bir_lowering=False)

    n_pad, units = _layout(n_sh)
    kinds, _ = _schedule(n_sh)
    n_cols = len(units)
    amin = mybir.AluOpType.min

    ypT_d = nc.declare_dram_parameter("ypT", [128, B], bf16, isOutput=False)
    YT_d = nc.declare_dram_parameter("YT", [128, n_sh], bf16, isOutput=False)
    accD_d = nc.declare_dram_parameter("accD", [128, n_cols], fp32, isOutput=True)

    with tile.TileContext(nc) as tc:
        with (
            tc.tile_pool(name="resident", bufs=1) as resident,
            tc.tile_pool(name="scr", bufs=2) as scrp,
            tc.tile_pool(name="psum", bufs=4, space="PSUM") as psump,
        ):
            yp_sb = resident.tile([128, B], bf16, tag="yp")
            # warm-up operand, memset on the earliest-ready engine
            # queue (gpsimd) so the warm-up matmuls start ~5us
            warm = resident.tile([128, 512], bf16, tag="warm")
            YT_sb = resident.tile([128, n_pad], bf16, tag="YT")
            accD = resident.tile([128, n_cols], fp32, tag="accD")
            bias_h = resident.tile([128, 1], fp32, tag="biash")

            nc.gpsimd.memset(warm[:], 0.0)
            # moving operand on the otherwise-idle scalar ring; gallery
            # stream entirely on the sync ring (ample bandwidth, and the
            # gpsimd ring's swdge generation + drains leave the critical
            # path), small chunks first so tile 0 lands ASAP
            nc.scalar.dma_start(yp_sb[:], ypT_d[:])
            s = 0
            widths = [128, 128, 256, 512, 1024, 2048] + [4096] * 8
            for w in widths:
                if s >= n_sh:
                    break
                w = min(w, n_sh - s)
                nc.sync.dma_start(YT_sb[:, s : s + w], YT_d[:, s : s + w])
                s += w
            if s < n_sh:
                nc.sync.dma_start(YT_sb[:, s:], YT_d[:, s:])
            if n_pad > n_sh:
                nc.vector.memset(YT_sb[:, n_sh:], 1.0)
            nc.vector.memset(bias_h[:], 0.5)
            nc.vector.memset(accD[:], 1.0)

            for pi, (t0, wt) in enumerate(units):
                x = psump.tile([P_TILE, 1024], fp32, tag="x")
                if pi == 0:
                    # PE warm-up during the preamble+DMA window: the
                    # p-state ramp (~4.7us) completes before real data
                    for _ in range(9):
                        nc.tensor.matmul(
                            x[:, :512], lhsT=warm[:, :128], rhs=warm[:],
                            start=True, stop=True,
                        )
                for h in range(wt):
                    t = t0 + h
                    ns = slice(t * P_TILE, (t + 1) * P_TILE)
                    nc.tensor.matmul(
                        x[:, h * 512 : (h + 1) * 512],
                        lhsT=YT_sb[:, ns], rhs=yp_sb[:],
                        start=True, stop=True,
                    )
                xa = x[:, : 512 * wt]
                col = accD[:, pi : pi + 1]
                if kinds[pi] == "ACT":
                    nc.scalar.activation(
                        xa, xa,
                        mybir.ActivationFunctionType.Relu,
                        bias=bias_h[:], scale=-1.0,
                        accum_out=col,
                    )
                else:
                    nc.vector.tensor_reduce(
                        col, xa, mybir.AxisListType.X, amin,
                    )

            cut = max(0, n_cols - 5)
            if cut:
                nc.sync.dma_start(accD_d[:, :cut], accD[:, :cut])
            nc.sync.dma_start(accD_d[:, cut:], accD[:, cut:])

    nc.finalize()
    return nc, n_cols


def _prep_host(u, y, ind, U, Y):
    """Scatter + device arrays (bf16) + fp64 base sum."""
    u = np.asarray(u, dtype=np.float32)
    y = np.asarray(y, dtype=np.float32)
    ind = np.asarray(ind).astype(np.int64)
    U2 = np.array(U, dtype=np.float32, copy=True)
    Y2 = np.array(Y, dtype=np.float32, copy=True)
    U2[ind] = u
    Y2[ind] = y

    u64 = u.astype(np.float64)
    U64 = U2.astype(np.float64)
    u_sq64 = (u64 * u64).sum(axis=1)
    U_sq64 = (U64 * U64).sum(axis=1)
    s_raw = (
        N * u_sq64.sum()
        + B * U_sq64.sum()
        - 2.0 * (u64.sum(axis=0) @ U64.sum(axis=0))
    )

    ypT = np.zeros((128, B), dtype=BF16)
    ypT[:C] = y.T.astype(BF16)
    YT = np.zeros((128, N), dtype=BF16)
    YT[:C] = Y2.T.astype(BF16)

    return u, y, U2, Y2, ypT, YT, s_raw


def _full_numpy_loss(u, y, U2, Y2):
    """Exact fp64 fallback (blocked); only used if detector preconditions
    fail (non-binary labels) -- never on spec inputs."""
    total = 0.0
    U64 = U2.astype(np.float64)
    Y64 = Y2.astype(np.float64)
    U_sq = (U64 * U64).sum(axis=1)
    for b0 in range(0, B, 64):
        ub = u[b0 : b0 + 64].astype(np.float64)
        yb = y[b0 : b0 + 64].astype(np.float64)
        dist = np.maximum(
            (ub * ub).sum(1)[:, None] - 2.0 * (ub @ U64.T) + U_sq[None, :], 0.0)
        mism = (yb @ Y64.T) == 0.0
        total += np.where(mism, 0.5 * np.maximum(M_MARGIN - dist, 0.0),
                          0.5 * dist).sum()
    loss1 = total / (B * N)
    loss2 = ALPHA * np.abs(1.0 - np.sign(u)).mean(dtype=np.float64)
    return np.array(loss1 + loss2, dtype=np.float32)


def _detector_preconditions_ok(y, Y2):
    return bool(((y == 0.0) | (y == 1.0)).all()
                and ((Y2 == 0.0) | (Y2 == 1.0)).all())


def _decode_flags(accD, n_sh):
    """Candidate local gallery rows. Col pi covers the tiles of unit
    pi; ACT cols flag > 0.25, DVE cols flag < 0.5."""
    kinds, units = _schedule(n_sh)
    cand = set()
    for pi, kind in enumerate(kinds):
        col = accD[:, pi]
        ps = np.nonzero(col > 0.25 if kind == "ACT" else col < 0.5)[0]
        t0, wt = units[pi]
        for p in ps:
            for j in range(wt):
                cand.add((t0 + j) * P_TILE + int(p))
    return sorted(n for n in cand if n < n_sh)


_PROG_CACHE = {}


def _get_program():
    key = ("v6", N_SH)
    if key not in _PROG_CACHE:
        _PROG_CACHE[key] = _build_program(N_SH)
    return _PROG_CACHE[key]


def kernel(u, y, ind, U, Y):
    u, y, U2, Y2, ypT, YT, s_raw = _prep_host(u, y, ind, U, Y)

    if not _detector_preconditions_ok(y, Y2):
        return _full_numpy_loss(u, y, U2, Y2)

    nc, n_cols = _get_program()
    in_maps = []
    for c in range(N_CORES):
        ns = slice(c * N_SH, (c + 1) * N_SH)
        in_maps.append({
            "ypT": ypT,
            "YT": np.ascontiguousarray(YT[:, ns]),
        })

    res = run_bass_kernel_spmd(nc, in_maps, list(range(N_CORES)))
    results = res.results

    y64 = y.astype(np.float64)
    corr = 0.0
    for c in range(N_CORES):
        accD = np.asarray(results[c]["accD"], dtype=np.float64)
        for n_loc in _decode_flags(accD, N_SH):
            n_glob = c * N_SH + n_loc
            match = y64 @ Y2[n_glob].astype(np.float64)
            for b in np.nonzero(match == 0.0)[0]:
                d = u[b].astype(np.float64) - U2[n_glob].astype(np.float64)
                raw = float(d @ d)
                corr += max(M_MARGIN - raw, 0.0) - raw

    total2 = s_raw + corr
    loss1 = 0.5 * total2 / (B * N)
    loss2 = ALPHA * np.abs(1.0 - np.sign(u)).mean(dtype=np.float64)
    return np.array(loss1 + loss2, dtype=np.float32)
